# revision 11
# baseline (speedup 1.0000x reference)
"""Trainium2 Bass kernel for nn_Decoder_Select (moe_routing).

Strategy: data-parallel over batch B=8 across 8 cores (1 batch sample
-> 2 stage-samples per core). The per-sample decoder dispatch
(ground_truth -> decoder index) is resolved on the HOST: each core
receives the weights of only the decoder its batch sample selects,
padded to n_src=3. The SPMD device program is identical on all cores;
only the input data differs. The padded 3rd source of decoder-0 cores
computes garbage which the host overwrites with zeros (matching the
reference's jnp.pad).

Device program per core (per stage-sample s in {0,1}):
  - selector: 2x 3x3 convs (dil 1 / dil 2) as 18 shifted matmuls over a
    zero-padded [128, 104x56] tile, 1x1 fusion conv, SE block, GAP, FC.
  - decoder: PReLU (max(a*x, x) on DVE), per-src 128->128 matmul,
    overlap-add fold as ONE strided DVE add (each output position is
    covered by exactly 2 chunks), tanh/sigmoid gate, mask matmul+relu,
    multiply by mixture_w, transposed-conv frames matmul (M=8 halves),
    final overlap-add (stride 8) as shifted copy + add, strided DMA out.

All matmuls run as float32r (full PE rate at N>=256, fp32 storage).
"""

import sys

import numpy as np

for _p in ("/opt/trn_rl_repo", "/root/.axon_site/_ro/trn_rl_repo"):
    if _p not in sys.path:
        sys.path.append(_p)

import concourse.bacc as bacc
import concourse.mybir as mybir
from concourse.bass_utils import run_bass_kernel_spmd
from concourse.tile import TileContext

F32 = mybir.dt.float32
FR = mybir.dt.float32r
AF = mybir.ActivationFunctionType
ALU = mybir.AluOpType
AX = mybir.AxisListType

B, S = 8, 2
BN, CK, NC = 128, 100, 52
IC, NF = 256, 2450
KS, ST, HOP = 16, 8, 50
N_SRCS = (2, 3)
MAXS = 3
T = KS + ST * (NF - 1)  # 19608
HW = CK * NC  # 5200
PH, PW = CK + 4, NC + 4  # padded conv input 104 x 56
NCORES = 8

# chunk plans: lists of (start, size); sizes >= 256 where possible so
# float32r matmuls run at full rate (1 cycle/row).
def _chunks(total, step=512, min_tail=256):
    out = []
    c = 0
    while c < total:
        n = min(step, total - c)
        rem = total - c
        if n < rem and rem - n < min_tail and rem <= 2 * step:
            # split remainder evenly to keep both chunks >= 256
            n = (rem + 1) // 2
        out.append((c, n))
        c += n
    return out

CH_HW = _chunks(HW)       # 5200 -> 9x512 + 296 + 296
CH_NF = _chunks(NF)       # 2450 -> 4x512 + 402
# conv row chunks: groups of h-rows, N = rows*52 (<=512 -> rows<=9)
CONV_ROWS = [9] * 10 + [5, 5]  # all N >= 260
NTAPS = 17

TAPS1 = [(dh, dw) for dh in (-1, 0, 1) for dw in (-1, 0, 1)]
TAPS2 = [(2 * dh, 2 * dw) for dh in (-1, 0, 1) for dw in (-1, 0, 1)]


def _build_program():
    nc = bacc.Bacc(None)

    # ---- I/O ----
    x_d = [
        nc.dram_tensor("x0", [BN, CK, NC], FR, kind="ExternalInput"),
        nc.dram_tensor("x1", [BN, CK, NC], FR, kind="ExternalInput"),
    ]
    mixw_d = nc.dram_tensor("mixw", [IC, NF], F32, kind="ExternalInput")
    cw_d = nc.dram_tensor("cw", [128, 17 * 64], FR, kind="ExternalInput")
    cb_d = nc.dram_tensor("cb", [64, 1], F32, kind="ExternalInput")
    fusw_d = nc.dram_tensor("fusw", [64, 64], FR, kind="ExternalInput")
    fusb_d = nc.dram_tensor("fusb", [64, 1], F32, kind="ExternalInput")
    se1_d = nc.dram_tensor("se1", [64, 16], F32, kind="ExternalInput")
    se2_d = nc.dram_tensor("se2", [16, 64], F32, kind="ExternalInput")
    fcw_d = nc.dram_tensor("fcw", [64, 2], F32, kind="ExternalInput")
    fcb_d = nc.dram_tensor("fcb", [2, 1], F32, kind="ExternalInput")
    fow_d = nc.dram_tensor("fow", [128, 3 * 128], FR, kind="ExternalInput")
    fob_d = nc.dram_tensor("fob", [128, 3], F32, kind="ExternalInput")
    now_d = nc.dram_tensor("now", [128, 128], FR, kind="ExternalInput")
    nob_d = nc.dram_tensor("nob", [128, 1], F32, kind="ExternalInput")
    ngw_d = nc.dram_tensor("ngw", [128, 128], FR, kind="ExternalInput")
    ngb_d = nc.dram_tensor("ngb", [128, 1], F32, kind="ExternalInput")
    mnw_d = nc.dram_tensor("mnw", [128, 256], FR, kind="ExternalInput")
    tcw_d = nc.dram_tensor("tcw", [128, 32], FR, kind="ExternalInput")
    alpha_d = nc.dram_tensor("alpha", [128, 1], F32, kind="ExternalInput")

    wavs_d = nc.dram_tensor("wavs", [S, MAXS, T], F32, kind="ExternalOutput")
    sel_d = nc.dram_tensor("sel", [S, 2], F32, kind="ExternalOutput")

    with TileContext(nc) as tc:
        with (
            tc.tile_pool(name="wp", bufs=1) as wp,
            tc.tile_pool(name="bp", bufs=1) as bp,
            tc.tile_pool(name="sp", bufs=2) as sp,
            tc.tile_pool(name="wvp", bufs=2) as wvp,
            tc.tile_pool(name="pp", bufs=6, space="PSUM") as pp,
        ):
            # ---- load weights ----
            def wtile(dram, shape, name):
                t = wp.tile(shape, dram.dtype, name=name)
                nc.sync.dma_start(t[:], dram[:])
                return t

            cw = wtile(cw_d, [128, 17 * 64], "cw")
            cb = wtile(cb_d, [64, 1], "cb")
            fusw = wtile(fusw_d, [64, 64], "fusw")
            fusb = wtile(fusb_d, [64, 1], "fusb")
            se1 = wtile(se1_d, [64, 16], "se1")
            se2 = wtile(se2_d, [16, 64], "se2")
            fcw = wtile(fcw_d, [64, 2], "fcw")
            fcb = wtile(fcb_d, [2, 1], "fcb")
            fow = wtile(fow_d, [128, 3 * 128], "fow")
            fob = wtile(fob_d, [128, 3], "fob")
            now = wtile(now_d, [128, 128], "now")
            nob = wtile(nob_d, [128, 1], "nob")
            ngw = wtile(ngw_d, [128, 128], "ngw")
            ngb = wtile(ngb_d, [128, 1], "ngb")
            mnw = wtile(mnw_d, [128, 256], "mnw")
            tcw = wtile(tcw_d, [128, 32], "tcw")
            alpha = wtile(alpha_d, [128, 1], "alpha")

            mixw = wp.tile([128, 2, NF], F32, name="mixw")
            nc.scalar.dma_start(
                mixw[:], mixw_d[:].rearrange("(b p) f -> p b f", p=128)
            )
            zt = wp.tile([128, 224], F32, name="zt")
            nc.vector.memset(zt[:], 0.0)
            z64 = wp.tile([64, 512], F32, name="z64")
            nc.vector.memset(z64[:], 0.0)

            def mm(out, lhsT, rhs, start=True, stop=True):
                nc.tensor.matmul(out, lhsT, rhs, start=start, stop=stop)

            for s in range(S):
                # ================= load & pad x =================
                xq = bp.tile([128, PH * PW], FR, tag="xq", name=f"xq{s}")
                xq3 = xq[:].rearrange("p (h w) -> p h w", h=PH)
                for bview in (
                    xq3[:, 0:2, :],
                    xq3[:, 2 + CK :, :],
                    xq3[:, 2 : 2 + CK, 0:2],
                    xq3[:, 2 : 2 + CK, 2 + NC :],
                ):
                    nz = bview.free_size()
                    nc.vector.tensor_copy(bview, zt[:, 0:nz])
                nc.sync.dma_start(xq3[:, 2 : 2 + CK, 2 : 2 + NC], x_d[s][:])

                # ================= selector =================
                h64 = bp.tile([64, HW], FR, tag="h64", name=f"h64_{s}")
                sq = bp.tile([64, 16], F32, tag="sq", name=f"sq{s}")
                h0 = 0
                for ci, rows in enumerate(CONV_ROWS):
                    n = rows * NC
                    cps = pp.tile([64, 512], F32, tag="mm", name=f"cps{s}_{ci}")
                    all_taps = [(0, 0)] + [t for t in TAPS1 if t != (0, 0)] + [
                        t for t in TAPS2 if t != (0, 0)
                    ]
                    for tap, (dh, dw) in enumerate(all_taps):
                        rhs = xq3[
                            :,
                            2 + dh + h0 : 2 + dh + h0 + rows,
                            2 + dw : 2 + dw + NC,
                        ]
                        mm(
                            cps[0:64, 0:n],
                            cw[:, tap * 64 : (tap + 1) * 64],
                            rhs,
                            start=(tap == 0),
                            stop=(tap == NTAPS - 1),
                        )
                    nc.scalar.activation(
                        h64[:, h0 * NC : h0 * NC + n],
                        cps[0:64, 0:n],
                        AF.Identity,
                        bias=cb[:, 0:1],
                    )
                    h0 += rows

                # 1x1 fusion conv (in-place into h64) + squeeze sums
                for ci, (c0, n) in enumerate(CH_HW):
                    fps = pp.tile([64, 512], F32, tag="mm", name=f"fps{s}_{ci}")
                    mm(fps[:, 0:n], fusw[:], h64[:, c0 : c0 + n])
                    nc.scalar.activation(
                        h64[:, c0 : c0 + n],
                        fps[0:64, 0:n],
                        AF.Identity,
                        bias=fusb[:, 0:1],
                        accum_out=sq[:, ci : ci + 1],
                    )

                sqs = sp.tile([64, 1], F32, tag="v64", name=f"sqs{s}")
                nc.vector.reduce_sum(sqs[:, 0:1], sq[:, 0 : len(CH_HW)], axis=AX.X)
                s1ps = pp.tile([16, 512], F32, tag="mm", name=f"s1ps{s}")
                nc.tensor.matmul(s1ps[0:16, 0:1], se1[:], sqs[:, 0:1], start=True, stop=True)
                s1 = sp.tile([16, 1], F32, tag="v16", name=f"s1_{s}")
                nc.scalar.activation(
                    s1[:], s1ps[0:16, 0:1], AF.Relu, scale=1.0 / HW
                )
                s2ps = pp.tile([64, 512], F32, tag="mm", name=f"s2ps{s}")
                nc.tensor.matmul(s2ps[0:64, 0:1], se2[:], s1[:], start=True, stop=True)
                svec = sp.tile([64, 1], F32, tag="v64", name=f"svec{s}")
                nc.scalar.activation(svec[:], s2ps[0:64, 0:1], AF.Sigmoid)

                pc = bp.tile([64, 16], F32, tag="pc", name=f"pc{s}")
                for ci, (c0, n) in enumerate(CH_HW):
                    htmp = sp.tile([64, 512], F32, tag="ta", name=f"ht{s}_{ci}")
                    nc.vector.scalar_tensor_tensor(
                        htmp[:, 0:n],
                        h64[:, c0 : c0 + n],
                        svec[:, 0:1],
                        z64[:, 0:n],
                        ALU.mult,
                        ALU.max,
                        accum_out=pc[:, ci : ci + 1],
                    )
                pooled = sp.tile([64, 1], F32, tag="v64", name=f"pool{s}")
                nc.vector.reduce_sum(pooled[:, 0:1], pc[:, 0 : len(CH_HW)], axis=AX.X)
                selps = pp.tile([2, 512], F32, tag="mm", name=f"selps{s}")
                nc.tensor.matmul(selps[0:2, 0:1], fcw[:], pooled[:, 0:1], start=True, stop=True)
                sel_sb = sp.tile([2, 1], F32, tag="v2", name=f"sel{s}")
                nc.scalar.activation(
                    sel_sb[:],
                    selps[0:2, 0:1],
                    AF.Identity,
                    scale=1.0 / HW,
                    bias=fcb[:, 0:1],
                )
                nc.sync.dma_start(
                    sel_d[s].rearrange("(a b) -> a b", b=1), sel_sb[:]
                )

                # ================= decoder =================
                # PReLU: max(alpha*x, x), reading x from the padded tile
                xp = bp.tile([128, HW], FR, tag="xp", name=f"xp{s}")
                xin = xq3[:, 2 : 2 + CK, 2 : 2 + NC]
                nc.vector.scalar_tensor_tensor(
                    xp[:].rearrange("p (h w) -> p h w", h=CK),
                    xin,
                    alpha[:, 0:1],
                    xin,
                    ALU.mult,
                    ALU.max,
                )

                # input-side fold (overlap-add commutes with the linear fo
                # matmul): xpf[b, 50m+q] = xp[b,(q+50)*52+m+1] + xp[b,q*52+m+2]
                xpf = bp.tile([128, NF], FR, tag="xpf", name=f"xpf{s}")
                xp_kj = xp[:].rearrange("p (k j) -> p j k", k=CK)
                nc.vector.tensor_add(
                    xpf[:].rearrange("p (m q) -> p m q", m=49),
                    xp_kj[:, 1:50, 50:100],
                    xp_kj[:, 2:51, 0:50],
                )

                for r in range(MAXS):
                    h = bp.tile([128, NF], FR, tag="h", bufs=2, name=f"h{s}_{r}")
                    o = bp.tile([128, NF], FR, tag="o", bufs=2, name=f"o{s}_{r}")
                    # fo matmul on folded input; bias is doubled host-side
                    for ci, (c0, n) in enumerate(CH_NF):
                        fps = pp.tile(
                            [128, 512], F32, tag="mm", name=f"fo{s}_{r}_{ci}"
                        )
                        mm(
                            fps[:, 0:n],
                            fow[:, r * 128 : (r + 1) * 128],
                            xpf[:, c0 : c0 + n],
                        )
                        nc.vector.tensor_scalar_add(
                            h[:, c0 : c0 + n], fps[:, 0:n], fob[:, r : r + 1]
                        )

                    # gated output: o = tanh(now@h + nob) * sigmoid(ngw@h + ngb)
                    for ci, (c0, n) in enumerate(CH_NF):
                        nps = pp.tile(
                            [128, 512], F32, tag="mm", name=f"no{s}_{r}_{ci}"
                        )
                        mm(nps[:, 0:n], now[:], h[:, c0 : c0 + n])
                        ta = sp.tile([128, 512], F32, tag="ta", name=f"ta{s}_{r}_{ci}")
                        nc.scalar.activation(
                            ta[:, 0:n], nps[:, 0:n], AF.Tanh, bias=nob[:, 0:1]
                        )
                        gps = pp.tile(
                            [128, 512], F32, tag="mm", name=f"ng{s}_{r}_{ci}"
                        )
                        mm(gps[:, 0:n], ngw[:], h[:, c0 : c0 + n])
                        tb = sp.tile([128, 512], F32, tag="tb", name=f"tb{s}_{r}_{ci}")
                        nc.scalar.activation(
                            tb[:, 0:n], gps[:, 0:n], AF.Sigmoid, bias=ngb[:, 0:1]
                        )
                        nc.gpsimd.tensor_mul(
                            o[:, c0 : c0 + n], ta[:, 0:n], tb[:, 0:n]
                        )

                    # mask -> *mixw -> frames -> overlap-add(stride 8)
                    wav = wvp.tile([8, NF + 1], F32, tag="wav", name=f"wv{s}_{r}")
                    wavB = wvp.tile([8, NF + 1], F32, tag="wavB", name=f"wb{s}_{r}", bufs=1)
                    for ci, (c0, n) in enumerate(CH_NF):
                        st = []
                        for oc in range(2):
                            mps = pp.tile(
                                [128, 512], F32, tag="mm",
                                name=f"mn{s}_{r}_{ci}_{oc}",
                            )
                            mm(
                                mps[:, 0:n],
                                mnw[:, oc * 128 : (oc + 1) * 128],
                                o[:, c0 : c0 + n],
                            )
                            mt = sp.tile(
                                [128, 512], F32, tag=f"mt{oc}",
                                name=f"mt{s}_{r}_{ci}_{oc}",
                            )
                            nc.vector.tensor_scalar_max(mt[:, 0:n], mps[:, 0:n], 0.0)
                            stt = sp.tile(
                                [128, 512], FR, tag=f"st{oc}",
                                name=f"st{s}_{r}_{ci}_{oc}",
                            )
                            nc.gpsimd.tensor_mul(
                                stt[:, 0:n], mt[:, 0:n], mixw[:, oc, c0 : c0 + n]
                            )
                            st.append(stt)
                        fra = pp.tile([8, 512], F32, tag="mm", name=f"fa{s}_{r}_{ci}")
                        mm(fra[:, 0:n], tcw[:, 0:8], st[0][:, 0:n], True, False)
                        mm(fra[:, 0:n], tcw[:, 8:16], st[1][:, 0:n], False, True)
                        frb = pp.tile([8, 512], F32, tag="mm", name=f"fb{s}_{r}_{ci}")
                        mm(frb[:, 0:n], tcw[:, 16:24], st[0][:, 0:n], True, False)
                        mm(frb[:, 0:n], tcw[:, 24:32], st[1][:, 0:n], False, True)
                        nc.scalar.activation(
                            wav[:, c0 : c0 + n], fra[0:8, 0:n], AF.Copy
                        )
                        nc.scalar.activation(
                            wavB[:, c0 + 1 : c0 + 1 + n], frb[0:8, 0:n], AF.Copy
                        )
                    nc.vector.memset(wav[:, NF : NF + 1], 0.0)
                    nc.vector.memset(wavB[:, 0:1], 0.0)
                    nc.vector.tensor_add(
                        wav[:, 0 : NF + 1], wav[:, 0 : NF + 1], wavB[:, 0 : NF + 1]
                    )
                    dma_eng = (nc.sync, nc.scalar)[(s * MAXS + r) % 2]
                    dma_eng.dma_start(
                        wavs_d[s, r].rearrange("(q c) -> c q", c=8),
                        wav[:, 0 : NF + 1],
                    )

    nc.finalize()
    return nc


_NC_CACHE = None


def _get_program():
    global _NC_CACHE
    if _NC_CACHE is None:
        _NC_CACHE = _build_program()
    return _NC_CACHE


def _prep_weights(params):
    """Host-side weight prep -> dict of per-decoder input tensors."""
    p = {k: np.asarray(v, dtype=np.float32) for k, v in params.items()}

    # selector weights (shared by all cores)
    cw = np.zeros((128, 17 * 64), np.float32)
    taps_all = [((0, 0), None)]
    t1 = [(dh, dw) for dh in (-1, 0, 1) for dw in (-1, 0, 1)]
    order = [(0, 0)] + [t for t in t1 if t != (0, 0)] + [
        (2 * dh, 2 * dw) for (dh, dw) in t1 if (dh, dw) != (0, 0)
    ]
    for g, wname in enumerate(("b1_w", "b2_w")):
        w = p[wname]  # [32, 128, 3, 3]
        for kh in range(3):
            for kw in range(3):
                d = (kh - 1, kw - 1) if g == 0 else (2 * (kh - 1), 2 * (kw - 1))
                tap = order.index(d) if d != (0, 0) else 0
                col = tap * 64 + g * 32
                cw[:, col : col + 32] = w[:, :, kh, kw].T
    sel_common = {
        "cw": cw,
        "cb": np.concatenate([p["b1_b"], p["b2_b"]])[:, None].copy(),
        "fusw": p["fus_w"][:, :, 0, 0].T.copy(),
        "fusb": p["fus_b"][:, None].copy(),
        "se1": p["se1"].T.copy(),
        "se2": p["se2"].T.copy(),
        "fcw": p["fc_w"].T.copy(),
        "fcb": p["fc_b"][:, None].copy(),
    }

    decs = []
    for d, n_src in enumerate(N_SRCS):
        fo_w = p[f"fo_w{d}"]  # [n_src*128, 128]
        fo_b = p[f"fo_b{d}"]
        fo_w3 = np.zeros((3 * 128, 128), np.float32)
        fo_w3[: n_src * 128] = fo_w
        fo_b3 = np.zeros((3 * 128,), np.float32)
        fo_b3[: n_src * 128] = fo_b
        tc_w = p[f"tc_w{d}"]  # [256, 16]
        tcw = np.concatenate(
            [tc_w[0:128, 0:8], tc_w[128:256, 0:8], tc_w[0:128, 8:16], tc_w[128:256, 8:16]],
            axis=1,
        )
        decs.append(
            {
                "fow": fo_w3.T.copy(),
                "fob": (2.0 * fo_b3).reshape(3, 128).T.copy(),
                "now": p[f"no_w{d}"].T.copy(),
                "nob": p[f"no_b{d}"][:, None].copy(),
                "ngw": p[f"ng_w{d}"].T.copy(),
                "ngb": p[f"ng_b{d}"][:, None].copy(),
                "mnw": p[f"mn_w{d}"].T.copy(),
                "tcw": np.ascontiguousarray(tcw),
                "alpha": np.full(
                    (128, 1), float(np.asarray(p[f"prelu{d}"])), np.float32
                ),
            }
        )
    return sel_common, decs


LAST_RESULT = None


def kernel(output0, output1, mixture_w, ground_truth, params):
    global LAST_RESULT
    output0 = np.ascontiguousarray(np.asarray(output0, dtype=np.float32))
    output1 = np.ascontiguousarray(np.asarray(output1, dtype=np.float32))
    mixture_w = np.ascontiguousarray(np.asarray(mixture_w, dtype=np.float32))
    gt = np.asarray(ground_truth)

    idx = np.argmax(gt[:, None] == np.array(N_SRCS, dtype=gt.dtype), axis=1)
    sel_common, decs = _prep_weights(params)

    nc = _get_program()
    in_maps = []
    for b in range(B):
        m = {
            "x0": output0[b],
            "x1": output1[b],
            "mixw": mixture_w[b],
        }
        m.update(sel_common)
        m.update(decs[int(idx[b])])
        in_maps.append(m)

    import os

    res = run_bass_kernel_spmd(
        nc,
        in_maps,
        list(range(NCORES)),
        trace=bool(os.environ.get("BASS_TRACE")),
    )
    LAST_RESULT = res

    output_wavs = np.zeros((B, S, MAXS, T), np.float32)
    sel = np.zeros((B, S, len(N_SRCS)), np.float32)
    for b in range(B):
        output_wavs[b] = res.results[b]["wavs"]
        if idx[b] == 0:
            output_wavs[b, :, N_SRCS[0] :, :] = 0.0
        sel[b] = res.results[b]["sel"]
    return output_wavs, sel
